# revision 14
# baseline (speedup 1.0000x reference)
"""Causal multi-head self-attention on 8 TRN2 NeuronCores.

Sharding: core c handles batch b = c//2 and head-half hh = c%2 (8 of 16
heads).  Each core computes qkv projection for its heads, RoPE, causal
attention, and a PARTIAL output projection (its heads' contribution to
Wout @ attn).  The host sums the two half-head partials per batch.
No collectives.

Attention uses TRANSPOSED scores sT[k, q] (no on-chip transposes); two
heads are row-packed into the PE array per score matmul; exp runs as one
ACT op per chunk-pair; attn@V uses a ones-column for the softmax sums
(head A: lhsT=[v|1] M=65, sums on partition 64; head B:
lhsT=[0(32)|1|0(31)|v] M=128, sums on partition 32, output on partitions
64-127 -- keeps every vector op lane-aligned).  Normalization: broadcast
the sums row with a K=1 ones-matmul, fast approximate reciprocal on the
broadcast, one tensor_mul per head fused with the psum->sbuf cast.

Scheduling: the attention inner loop is ACT(exp)-paced (~1147ns per
chunk vs ~640ns of PE matmul work), so ALL projection + output-
projection work is split into ~2-matmul "pieces" and pumped one piece
per attention chunk (between the scores pair and the lagged attn@V, so
every LDWEIGHTS hides under a live matmul), keeping the in-order PE
queue dense.  Each pair's attn@V tail (3-chunk exp lag) is drained
AFTER the next pair's projection burst so it never stalls the PE; at
short early pairs extra filler bridges the exp catch-up.  Input DMA is
d-major with 2-3KB packet lines, split in sequence halves so qb0/qb1's
working set (~5MB) lands first; cosq/sinq are derived on-chip from
cosk/sink (exact x0.125 scale).  A short warm-up matmul run bridges
HAM to 2.4GHz at start, and warm-keeper matmuls span the final norm
chain so the tail outproj doesn't run at the cold clock.
"""

import numpy as np
import ml_dtypes
from collections import deque

BF16 = ml_dtypes.bfloat16

B, S, D = 4, 2048, 1024
H, DK = 16, 64
THETA = 10000.0
NCORES = 8
HLOC = H // 2
NPAIR = HLOC // 2
P = 128
SBLK = 512
OV = HLOC * DK
WCOLS = 3 * OV


def _rope_perm():
    return np.concatenate([np.arange(0, DK, 2), np.arange(1, DK, 2)])


def _host_tables(s):
    half = DK // 2
    inv_freq = THETA ** (-np.arange(0, DK, 2, dtype=np.float64) / DK)
    pos = np.arange(s, dtype=np.float64)
    ang = pos[None, :] * inv_freq[:, None]
    c, sn = np.cos(ang), np.sin(ang)
    cos_t = np.empty((P, s), np.float32)
    sin_t = np.empty((P, s), np.float32)
    for hrow in range(2):
        o = hrow * DK
        cos_t[o:o + half] = c
        cos_t[o + half:o + DK] = c
        sin_t[o:o + half] = -sn
        sin_t[o + half:o + DK] = sn
    scale = 1.0 / np.sqrt(DK)
    cosq = (cos_t * scale).astype(BF16)
    sinq = (sin_t * scale).astype(BF16)
    cosk = cos_t.astype(BF16)
    sink = sin_t.astype(BF16)

    swap = np.zeros((P, P), np.float32)
    for hrow in range(2):
        o = hrow * DK
        for i in range(half):
            swap[o + i, o + half + i] = 1.0
            swap[o + half + i, o + i] = 1.0
    swapm = swap.astype(BF16)

    tri = (np.arange(P)[:, None] <= np.arange(P)[None, :]).astype(np.float32)
    tri2 = np.concatenate([tri, tri], axis=1).astype(BF16)
    return cosq, sinq, cosk, sink, swapm, tri2


def _build_nc(s=S):
    import concourse.bass as bass  # noqa: F401
    import concourse.mybir as mybir
    import concourse.tile as tile
    from concourse import bacc
    from contextlib import ExitStack

    f32 = mybir.dt.float32
    bf16 = mybir.dt.bfloat16
    EXP = mybir.ActivationFunctionType.Exp
    MUL = mybir.AluOpType.mult

    nsb = s // SBLK
    dch = D // P
    assert nsb % 2 == 0

    nc = bacc.Bacc(None, target_bir_lowering=False)
    xT_d = nc.dram_tensor("xT", [D, s], bf16, kind="ExternalInput")
    wq_d = nc.dram_tensor("wqkvT", [D, WCOLS], bf16, kind="ExternalInput")
    wo_d = nc.dram_tensor("woutT", [OV, D], bf16, kind="ExternalInput")
    cosk_d = nc.dram_tensor("cosk", [P, s], bf16, kind="ExternalInput")
    sink_d = nc.dram_tensor("sink", [P, s], bf16, kind="ExternalInput")
    swap_d = nc.dram_tensor("swapm", [P, P], bf16, kind="ExternalInput")
    tri_d = nc.dram_tensor("tri2", [P, 2 * P], bf16, kind="ExternalInput")
    out_d = nc.dram_tensor("out", [s, D], f32, kind="ExternalOutput")

    W2 = 2 * SBLK

    with tile.TileContext(nc) as tc, ExitStack() as ctx:
        const = ctx.enter_context(tc.tile_pool(name="const", bufs=1))
        psS = ctx.enter_context(
            tc.tile_pool(name="psS", bufs=2, space="PSUM"))
        psB = ctx.enter_context(
            tc.tile_pool(name="psB", bufs=4, space="PSUM"))
        rpool = ctx.enter_context(tc.tile_pool(name="rope", bufs=2))
        ppool = ctx.enter_context(tc.tile_pool(name="probs", bufs=8))
        npool = ctx.enter_context(tc.tile_pool(name="norm", bufs=2))
        opool = ctx.enter_context(tc.tile_pool(name="outsb", bufs=4))
        atpool = ctx.enter_context(tc.tile_pool(name="attnT", bufs=2))

        # ---- input DMA; per-d interleave of (wq[d], xT[d] first quarter) so
        # the first projection units can start after ~2 of 8 d-chunks land ---
        hs = s // 2
        qs_ = s // 4
        wq = []
        xT = []
        for d in range(dch):
            t = const.tile([P, WCOLS], bf16, tag=f"wq{d}")
            nc.sync.dma_start(out=t, in_=wq_d[d * P:(d + 1) * P, :])
            wq.append(t)
            t = const.tile([P, s], bf16, tag=f"xT{d}")
            nc.sync.dma_start(out=t[:, 0:qs_],
                              in_=xT_d[d * P:(d + 1) * P, 0:qs_])
            xT.append(t)
        # HAM warm-up: starts immediately (no DMA deps) and bridges PE
        # activity until the first input DMAs land
        warm = const.tile([P, SBLK], bf16, tag="warm")
        nc.vector.memset(warm, 1.0)
        wps = psB.tile([P, SBLK], f32, tag="acc")
        for _ in range(18):
            nc.tensor.matmul(wps, warm[:, 0:P], warm,
                             start=True, stop=True)

        tabs = {}
        for nm, dram in (("cosk", cosk_d), ("sink", sink_d)):
            t = const.tile([P, s], bf16, tag=nm)
            nc.sync.dma_start(out=t[:, 0:hs], in_=dram[:, 0:hs])
            tabs[nm] = t
        for nm, srcnm in (("cosq", "cosk"), ("sinq", "sink")):
            t = const.tile([P, s], bf16, tag=nm)
            nc.vector.tensor_scalar_mul(
                t[:, 0:hs], tabs[srcnm][:, 0:hs], 1.0 / np.sqrt(DK))
            tabs[nm] = t
        swap_sb = const.tile([P, P], bf16, tag="swapm")
        nc.sync.dma_start(out=swap_sb, in_=swap_d[:, :])
        tri_sb = const.tile([P, 2 * P], bf16, tag="tri2")
        nc.sync.dma_start(out=tri_sb, in_=tri_d[:, :])
        for d in range(dch):
            nc.sync.dma_start(out=xT[d][:, qs_:hs],
                              in_=xT_d[d * P:(d + 1) * P, qs_:hs])
        for d in range(dch):
            nc.sync.dma_start(out=xT[d][:, hs:s],
                              in_=xT_d[d * P:(d + 1) * P, hs:s])
        for nm, dram in (("cosk", cosk_d), ("sink", sink_d)):
            nc.sync.dma_start(out=tabs[nm][:, hs:s], in_=dram[:, hs:s])
        for nm, srcnm in (("cosq", "cosk"), ("sinq", "sink")):
            nc.vector.tensor_scalar_mul(
                tabs[nm][:, hs:s], tabs[srcnm][:, hs:s], 1.0 / np.sqrt(DK))
        wo = []
        for i in range(OV // P):
            t = const.tile([P, D], bf16, tag=f"wo{i}")
            nc.sync.dma_start(out=t, in_=wo_d[i * P:(i + 1) * P, :])
            wo.append(t)
        tri3 = tri_sb.rearrange("p (h q) -> p h q", h=2)
        ones_sb = const.tile([P, DK], bf16, tag="ones")
        nc.vector.memset(ones_sb, 1.0)
        # pre-allocate vA/vB tiles; their static regions (ones column for
        # the softmax sums, zero padding) are set once here instead of in
        # every v-unit fin
        va_t = []
        vb_t = []
        for sc in range(s // P):
            va = const.tile([P, NPAIR, 65], bf16, tag=f"vA{sc}",
                            name=f"vA{sc}")
            vb = const.tile([P, NPAIR, P], bf16, tag=f"vB{sc}",
                            name=f"vB{sc}")
            nc.gpsimd.memset(va[:, :, DK:DK + 1], 1.0)
            nc.gpsimd.memset(vb[:, :, 0:32], 0.0)
            nc.gpsimd.memset(vb[:, :, 32:33], 1.0)
            nc.gpsimd.memset(vb[:, :, 33:DK], 0.0)
            va_t.append(va)
            vb_t.append(vb)

        # ---- projection / outproj units, split into ~2-matmul pieces --------
        qt = [[None] * (nsb // 2) for _ in range(NPAIR)]   # [128, 1024]
        kt = [[None] * (nsb // 2) for _ in range(NPAIR)]
        vA = [[None] * NPAIR for _ in range(s // P)]
        vB = [[None] * NPAIR for _ in range(s // P)]

        unit_pieces = {}   # key -> deque of closures
        order = deque()    # unit keys in demand order

        def reg_unit(key, plist):
            if key in unit_pieces:
                return
            unit_pieces[key] = deque(plist)
            order.append(key)

        def qk_unit(is_q, pr, sbp, half):
            st = {}
            dests = qt if is_q else kt
            nm = ("qt" if is_q else "kt") + f"{pr}_{sbp}"
            wcol = (pr if is_q else NPAIR + pr) * P
            ct = tabs["cosq"] if is_q else tabs["cosk"]
            stt = tabs["sinq"] if is_q else tabs["sink"]
            cb = 2 * sbp + half
            c0 = cb * SBLK

            def mm(d0):
                def go():
                    if d0 == 0:
                        st['ps'] = psB.tile([P, SBLK], f32, tag="acc", name=f"ps{nm}")
                    for d in (d0, d0 + 1):
                        nc.tensor.matmul(
                            st['ps'], wq[d][:, wcol:wcol + P],
                            xT[d][:, c0:c0 + SBLK],
                            start=(d == 0), stop=(d == dch - 1))
                return go

            def fin():
                if dests[pr][sbp] is None:
                    dests[pr][sbp] = const.tile(
                        [P, W2], bf16, tag=nm, name=nm)
                dest = dests[pr][sbp]
                ps = st['ps']
                y = rpool.tile([P, SBLK], bf16, tag="y")
                nc.vector.tensor_copy(y, ps)
                sw = psB.tile([P, SBLK], f32, tag="acc")
                nc.tensor.matmul(sw, swap_sb, y, start=True, stop=True)
                t1 = rpool.tile([P, SBLK], bf16, tag="t1")
                nc.vector.tensor_mul(t1, y, ct[:, c0:c0 + SBLK])
                t2 = rpool.tile([P, SBLK], bf16, tag="t2")
                nc.vector.tensor_mul(t2, sw, stt[:, c0:c0 + SBLK])
                sl = slice(half * SBLK, (half + 1) * SBLK)
                nc.vector.tensor_add(dest[:, sl], t1, t2)

            return [mm(0), mm(2), mm(4), mm(6), fin]

        def v_unit(sc):
            st = {}

            def mm(d0):
                def go():
                    if d0 == 0:
                        st['ps'] = psB.tile([P, OV], f32, tag="acc", name=f"psv{sc}")
                    for d in (d0, d0 + 1):
                        nc.tensor.matmul(
                            st['ps'],
                            xT[d][:, sc * P:(sc + 1) * P],
                            wq[d][:, 2 * OV:],
                            start=(d == 0), stop=(d == dch - 1))
                return go

            def fin():
                psv = st['ps'].rearrange(
                    "p (a two d) -> p a two d", two=2, d=DK)
                va = va_t[sc]
                vb = vb_t[sc]
                nc.vector.tensor_copy(va[:, :, 0:DK], psv[:, :, 0, :])
                nc.vector.tensor_copy(vb[:, :, DK:2 * DK], psv[:, :, 1, :])
                for pr in range(NPAIR):
                    vA[sc][pr] = va[:, pr, :]
                    vB[sc][pr] = vb[:, pr, :]

            return [mm(0), mm(2), mm(4), mm(6), fin]

        def op_unit(qb, qc, nb):
            st = {}
            q_glob = qb * SBLK + qc * P
            nsl = slice(nb * SBLK, (nb + 1) * SBLK)

            def mm(pr0):
                def go():
                    if pr0 == 0:
                        st['po'] = psB.tile([P, SBLK], f32, tag="acc", name=f"po{qb}_{qc}_{nb}")
                    for pr2 in (pr0, pr0 + 1):
                        nc.tensor.matmul(
                            st['po'],
                            at[qb][pr2][:, qc * P:(qc + 1) * P],
                            wo[pr2][:, nsl],
                            start=(pr2 == 0), stop=(pr2 == NPAIR - 1))
                return go

            def fin():
                osb = opool.tile([P, SBLK], f32, tag="osb")
                # alternate the psum->sbuf copy between DVE and ACT so the
                # final outproj drain isn't serialized on one engine
                if (qc + nb) % 2 == 0:
                    nc.vector.tensor_copy(osb, st['po'])
                else:
                    nc.scalar.copy(osb, st['po'])
                nc.sync.dma_start(out=out_d[q_glob:q_glob + P, nsl], in_=osb)

            return [mm(0), mm(2), fin]

        # demand-ordered registration of all projection units; v units are
        # registered one qb ahead so their DVE fins land well before the
        # first attnV matmul that streams them
        for qb in range(nsb):
            for sc in range(4 * qb, min(4 * qb + 8, s // P)):
                reg_unit(("v", sc), v_unit(sc))
            for pr in range(NPAIR):
                reg_unit(("q", pr, qb // 2, qb % 2),
                         qk_unit(True, pr, qb // 2, qb % 2))
                for kcb in range(qb + 1):
                    reg_unit(("k", pr, kcb // 2, kcb % 2),
                             qk_unit(False, pr, kcb // 2, kcb % 2))

        def pump_one():
            while order:
                key = order[0]
                dq = unit_pieces.get(key)
                if not dq:
                    order.popleft()
                    unit_pieces.pop(key, None)
                    continue
                dq.popleft()()
                if not dq:
                    order.popleft()
                    del unit_pieces[key]
                return True
            return False

        def run_unit(key):
            dq = unit_pieces.get(key)
            if dq is None:
                return
            # finish any half-open unit first so at most one proj psum
            # accumulator is live at a time
            while order and order[0] != key:
                fk = order[0]
                fdq = unit_pieces.get(fk)
                if fdq and len(fdq) < len_total.get(fk, 99):
                    while fdq:
                        fdq.popleft()()
                    order.popleft()
                    del unit_pieces[fk]
                else:
                    break
            while dq:
                dq.popleft()()
            unit_pieces.pop(key, None)
            if order and order[0] == key:
                order.popleft()

        len_total = {k: len(v) for k, v in unit_pieces.items()}

        def qt_sl(pr, qb, lo, hi, rows):
            t = qt[pr][qb // 2]
            off = (qb % 2) * SBLK
            return t[rows[0]:rows[1]][:, off + lo:off + hi]

        def kt_sl(pr, kc, r0, r1):
            t = kt[pr][kc // 8]
            off = (kc % 8) * P
            return t[r0:r1, off:off + P]

        # ---- attention ------------------------------------------------------
        at = [[None] * NPAIR for _ in range(nsb)]

        def emit_norm(qb, pr, accA, accB):
            rtb = npool.tile([P, SBLK], bf16, tag="recipb")
            with nc.allow_low_precision(reason="bf16 softmax denom"):
                nc.vector.tensor_copy(
                    rtb[DK:DK + 1, :], accA[DK:DK + 1, :])
                nc.vector.tensor_copy(rtb[32:33, :], accB[32:33, :])
            # plain-MM filler so the broadcast matmuls below never wait on
            # the DVE copies at the PE queue head
            pump_one()
            pump_one()
            rbp = psS.tile([P, W2], f32, tag="mm")
            nc.tensor.matmul(
                rbp[0:DK, 0:SBLK], ones_sb[DK:DK + 1, :],
                rtb[DK:DK + 1, :],
                start=True, stop=True, tile_position=(64, 0))
            nc.tensor.matmul(
                rbp[DK:P, 0:SBLK], ones_sb[32:33, :], rtb[32:33, :],
                start=True, stop=True, tile_position=(32, 64))
            rbs = npool.tile([P, SBLK], f32, tag="rbcast")
            nc.vector.reciprocal_approx_fast(rbs, rbp[:, 0:SBLK])
            atile = atpool.tile([P, SBLK], bf16, tag=f"at{pr}")
            nc.vector.tensor_tensor(
                atile[0:DK, :], accA[0:DK, :], rbs[0:DK, :], op=MUL)
            nc.vector.tensor_tensor(
                atile[DK:P, :], accB[DK:P, :], rbs[DK:P, :], op=MUL)
            at[qb][pr] = atile

        def drain2(pending, accA, accB, pr, nkc):
            # drain 2 pending chunks' attnV (4 plain MMs, adjacent)
            for _ in range(2):
                pkc, ppp, pq0 = pending.pop(0)
                nc.tensor.matmul(
                    accA[0:65, pq0:SBLK], vA[pkc][pr],
                    ppp[:, pq0:SBLK],
                    start=(pkc == 0), stop=(pkc == nkc - 1))
                nc.tensor.matmul(
                    accB[0:P, pq0:SBLK], vB[pkc][pr],
                    ppp[:, SBLK + pq0:W2],
                    start=(pkc == 0), stop=(pkc == nkc - 1))

        def finish_pair(pend):
            # drain the deferred attnV tail (exp caught up during the burst
            # above), then normalize; register outproj once a qb completes
            pqb, ppr, paccA, paccB, ppending = pend
            pnkc = 4 * pqb + 4
            while ppending:
                drain2(ppending, paccA, paccB, ppr, pnkc)
            emit_norm(pqb, ppr, paccA, paccB)
            if ppr == NPAIR - 1:
                for qc in range(SBLK // P):
                    for nb in range(D // SBLK):
                        key = ("o", pqb, qc, nb)
                        unit_pieces[key] = deque(op_unit(pqb, qc, nb))
                        len_total[key] = len(unit_pieces[key])
                        order.appendleft(key)

        pend = None
        for qb in range(nsb):
            for pr in range(NPAIR):
                # ensure this pair's projection inputs are complete
                for sc in range(4 * qb, 4 * qb + 4):
                    run_unit(("v", sc))
                run_unit(("q", pr, qb // 2, qb % 2))
                for kcb in range(qb + 1):
                    run_unit(("k", pr, kcb // 2, kcb % 2))
                if pend is not None:
                    if qb < 2:
                        # short pairs: bridge the exp-catchup latency with
                        # filler so the drain below never stalls the PE
                        for _ in range(3):
                            pump_one()
                    finish_pair(pend)
                    pend = None
                accA = psB.tile([P, SBLK], f32, tag="acc")
                accB = psB.tile([P, SBLK], f32, tag="acc")
                nkc = 4 * qb + 4
                pending = []
                # 2-chunk groups: scores pairs adjacent (one tiled burst),
                # then exps, then plain pump + lagged attnV (plain burst) --
                # halves the PE tiled<->plain reconfig bubbles
                for g in range(nkc // 2):
                    descs = []
                    for c in (0, 1):
                        kc = 2 * g + c
                        diag_o = kc - 4 * qb
                        q0 = max(diag_o, 0) * P
                        sp = psS.tile([P, W2], f32, tag="mm")
                        nc.tensor.matmul(
                            sp[:, q0:SBLK],
                            kt_sl(pr, kc, 0, DK),
                            qt_sl(pr, qb, q0, SBLK, (0, DK)),
                            start=True, stop=True, tile_position=(0, 0))
                        nc.tensor.matmul(
                            sp[:, SBLK + q0:W2],
                            kt_sl(pr, kc, DK, P),
                            qt_sl(pr, qb, q0, SBLK, (DK, P)),
                            start=True, stop=True, tile_position=(64, 0))
                        descs.append((kc, sp, q0, diag_o))
                    for kc, sp, q0, diag_o in descs:
                        pp = ppool.tile([P, W2], bf16, tag="p")
                        if diag_o < 0:
                            nc.scalar.activation(pp, sp, EXP)
                        else:
                            nc.scalar.activation(
                                pp.rearrange(
                                    "p (h q) -> p h q", h=2)[:, :, q0:SBLK],
                                sp.rearrange(
                                    "p (h q) -> p h q", h=2)[:, :, q0:SBLK],
                                EXP)
                        if diag_o >= 0:
                            sl = pp.rearrange(
                                "p (h q) -> p h q", h=2)[:, :, q0:q0 + P]
                            nc.vector.tensor_tensor(sl, sl, tri3, op=MUL)
                        pending.append((kc, pp, q0))
                    # plain filler between the tiled scores burst and the
                    # attnV burst; keeps LDWEIGHTS hidden + exp time covered
                    pump_one()
                    pump_one()
                    if len(pending) >= 8:
                        drain2(pending, accA, accB, pr, nkc)
                pend = (qb, pr, accA, accB, pending)
        # keep the PE clock warm across the final norm chain latency
        wps2 = psB.tile([P, SBLK], f32, tag="acc", name="warmtail")
        for _ in range(6):
            nc.tensor.matmul(wps2, warm[:, 0:P], warm,
                             start=True, stop=True)
        finish_pair(pend)
        for _ in range(8):
            nc.tensor.matmul(wps2, warm[:, 0:P], warm,
                             start=True, stop=True)
        while pump_one():
            pass

    nc.finalize()
    return nc


def _host_prep(x, Wqkv, Wout, s=S):
    perm = _rope_perm()
    cosq, sinq, cosk, sink, swapm, tri2 = _host_tables(s)
    in_maps = []
    for c in range(NCORES):
        b, hh = c // 2, c % 2
        rows = []
        for sect in range(3):
            base = sect * D + hh * OV
            for h in range(HLOC):
                r = base + h * DK + (perm if sect < 2 else np.arange(DK))
                rows.append(r)
        idx = np.concatenate(rows)
        wslice = Wqkv[idx, :]
        in_maps.append({
            "xT": np.ascontiguousarray(x[b].T).astype(BF16),
            "wqkvT": np.ascontiguousarray(wslice.T).astype(BF16),
            "woutT": np.ascontiguousarray(
                Wout[:, hh * OV:(hh + 1) * OV].T).astype(BF16),
            "cosk": cosk, "sink": sink,
            "swapm": swapm, "tri2": tri2,
        })
    return in_maps


def kernel(x, Wqkv, Wout):
    from concourse.bass_utils import run_bass_kernel_spmd

    x = np.asarray(x, dtype=np.float32)
    Wqkv = np.asarray(Wqkv, dtype=np.float32)
    Wout = np.asarray(Wout, dtype=np.float32)

    nc = _build_nc(S)
    in_maps = _host_prep(x, Wqkv, Wout, S)
    res = run_bass_kernel_spmd(nc, in_maps, core_ids=list(range(NCORES)))
    outs = res.results
    out = np.empty((B, S, D), np.float32)
    for b in range(B):
        out[b] = outs[2 * b]["out"] + outs[2 * b + 1]["out"]
    return out



# revision 21
# speedup vs baseline: 1.0189x; 1.0189x over previous
"""Causal multi-head self-attention on 8 TRN2 NeuronCores.

Sharding: core c handles batch b = c//2 and head-half hh = c%2 (8 of 16
heads).  Each core computes qkv projection for its heads, RoPE, causal
attention, and a PARTIAL output projection (its heads' contribution to
Wout @ attn).  The host sums the two half-head partials per batch.
No collectives.

Attention uses TRANSPOSED scores sT[k, q] (no on-chip transposes); two
heads are row-packed into the PE array per score matmul; exp runs as one
ACT op per chunk-pair; attn@V uses a ones-column for the softmax sums
(head A: lhsT=[v|1] M=65, sums on partition 64; head B:
lhsT=[0(32)|1|0(31)|v] M=128, sums on partition 32, output on partitions
64-127 -- keeps every vector op lane-aligned).  Normalization: broadcast
the sums row with a K=1 ones-matmul, fast approximate reciprocal on the
broadcast, one tensor_mul per head fused with the psum->sbuf cast.

Scheduling: the attention inner loop is ACT(exp)-paced (~1147ns per
chunk vs ~640ns of PE matmul work), so ALL projection + output-
projection work is split into ~2-matmul "pieces" and pumped one piece
per attention chunk (between the scores pair and the lagged attn@V, so
every LDWEIGHTS hides under a live matmul), keeping the in-order PE
queue dense.  Each pair's attn@V tail (3-chunk exp lag) is drained
AFTER the next pair's projection burst so it never stalls the PE; at
short early pairs extra filler bridges the exp catch-up.  Input DMA is
d-major with 2-3KB packet lines, split in sequence halves so qb0/qb1's
working set (~5MB) lands first; cosq/sinq are derived on-chip from
cosk/sink (exact x0.125 scale).  A short warm-up matmul run bridges
HAM to 2.4GHz at start, and warm-keeper matmuls span the final norm
chain so the tail outproj doesn't run at the cold clock.
"""

import numpy as np
import ml_dtypes
from collections import deque

BF16 = ml_dtypes.bfloat16

B, S, D = 4, 2048, 1024
H, DK = 16, 64
THETA = 10000.0
NCORES = 8
HLOC = H // 2
NPAIR = HLOC // 2
P = 128
SBLK = 512
OV = HLOC * DK
WCOLS = 3 * OV


def _rope_perm():
    return np.concatenate([np.arange(0, DK, 2), np.arange(1, DK, 2)])


def _host_tables(s):
    half = DK // 2
    inv_freq = THETA ** (-np.arange(0, DK, 2, dtype=np.float64) / DK)
    pos = np.arange(s, dtype=np.float64)
    ang = pos[None, :] * inv_freq[:, None]
    c, sn = np.cos(ang), np.sin(ang)
    cos_t = np.empty((P, s), np.float32)
    sin_t = np.empty((P, s), np.float32)
    for hrow in range(2):
        o = hrow * DK
        cos_t[o:o + half] = c
        cos_t[o + half:o + DK] = c
        sin_t[o:o + half] = -sn
        sin_t[o + half:o + DK] = sn
    scale = 1.0 / np.sqrt(DK)
    cosq = (cos_t * scale).astype(BF16)
    sinq = (sin_t * scale).astype(BF16)
    cosk = cos_t.astype(BF16)
    sink = sin_t.astype(BF16)

    swap = np.zeros((P, P), np.float32)
    for hrow in range(2):
        o = hrow * DK
        for i in range(half):
            swap[o + i, o + half + i] = 1.0
            swap[o + half + i, o + i] = 1.0
    swapm = swap.astype(BF16)

    tri = (np.arange(P)[:, None] <= np.arange(P)[None, :]).astype(np.float32)
    tri2 = np.concatenate([tri, tri], axis=1).astype(BF16)
    return cosq, sinq, cosk, sink, swapm, tri2


def _build_nc(s=S):
    import concourse.bass as bass  # noqa: F401
    import concourse.mybir as mybir
    import concourse.tile as tile
    from concourse import bacc
    from contextlib import ExitStack

    f32 = mybir.dt.float32
    bf16 = mybir.dt.bfloat16
    EXP = mybir.ActivationFunctionType.Exp
    MUL = mybir.AluOpType.mult

    nsb = s // SBLK
    dch = D // P
    assert nsb % 2 == 0

    nc = bacc.Bacc(None, target_bir_lowering=False)
    xT_d = nc.dram_tensor("xT", [D, s], bf16, kind="ExternalInput")
    wq_d = nc.dram_tensor("wqkvT", [D, WCOLS], bf16, kind="ExternalInput")
    wo_d = nc.dram_tensor("woutT", [OV, D], bf16, kind="ExternalInput")
    cosk_d = nc.dram_tensor("cosk", [P, s], bf16, kind="ExternalInput")
    sink_d = nc.dram_tensor("sink", [P, s], bf16, kind="ExternalInput")
    swap_d = nc.dram_tensor("swapm", [P, P], bf16, kind="ExternalInput")
    tri_d = nc.dram_tensor("tri2", [P, 2 * P], bf16, kind="ExternalInput")
    out_d = nc.dram_tensor("out", [s, D], f32, kind="ExternalOutput")

    W2 = 2 * SBLK

    with tile.TileContext(nc) as tc, ExitStack() as ctx:
        const = ctx.enter_context(tc.tile_pool(name="const", bufs=1))
        psS = ctx.enter_context(
            tc.tile_pool(name="psS", bufs=2, space="PSUM"))
        psB = ctx.enter_context(
            tc.tile_pool(name="psB", bufs=4, space="PSUM"))
        rpool = ctx.enter_context(tc.tile_pool(name="rope", bufs=2))
        ppool = ctx.enter_context(tc.tile_pool(name="probs", bufs=6))
        npool = ctx.enter_context(tc.tile_pool(name="norm", bufs=2))
        opool = ctx.enter_context(tc.tile_pool(name="outsb", bufs=4))
        atpool = ctx.enter_context(tc.tile_pool(name="attnT", bufs=2))

        # ---- input DMA; per-d interleave of (wq[d], xT[d] first quarter) so
        # the first projection units can start after ~2 of 8 d-chunks land ---
        hs = s // 2
        qs_ = s // 4
        wq = []
        xT = []
        for d in range(dch):
            t = const.tile([P, WCOLS], bf16, tag=f"wq{d}")
            nc.sync.dma_start(out=t, in_=wq_d[d * P:(d + 1) * P, :])
            wq.append(t)
            t = const.tile([P, s], bf16, tag=f"xT{d}")
            nc.sync.dma_start(out=t[:, 0:qs_],
                              in_=xT_d[d * P:(d + 1) * P, 0:qs_])
            xT.append(t)
        tabs = {}
        for nm, dram in (("cosk", cosk_d), ("sink", sink_d)):
            t = const.tile([P, s], bf16, tag=nm)
            nc.sync.dma_start(out=t[:, 0:hs], in_=dram[:, 0:hs])
            tabs[nm] = t
        for nm, srcnm in (("cosq", "cosk"), ("sinq", "sink")):
            t = const.tile([P, s], bf16, tag=nm)
            nc.vector.tensor_scalar_mul(
                t[:, 0:hs], tabs[srcnm][:, 0:hs], 1.0 / np.sqrt(DK))
            tabs[nm] = t
        swap_sb = const.tile([P, P], bf16, tag="swapm")
        nc.sync.dma_start(out=swap_sb, in_=swap_d[:, :])
        tri_sb = const.tile([P, 2 * P], bf16, tag="tri2")
        nc.sync.dma_start(out=tri_sb, in_=tri_d[:, :])
        for d in range(dch):
            nc.sync.dma_start(out=xT[d][:, qs_:hs],
                              in_=xT_d[d * P:(d + 1) * P, qs_:hs])
        for d in range(dch):
            nc.sync.dma_start(out=xT[d][:, hs:s],
                              in_=xT_d[d * P:(d + 1) * P, hs:s])
        for nm, dram in (("cosk", cosk_d), ("sink", sink_d)):
            nc.sync.dma_start(out=tabs[nm][:, hs:s], in_=dram[:, hs:s])
        for nm, srcnm in (("cosq", "cosk"), ("sinq", "sink")):
            nc.vector.tensor_scalar_mul(
                tabs[nm][:, hs:s], tabs[srcnm][:, hs:s], 1.0 / np.sqrt(DK))
        wo = []
        for i in range(OV // P):
            t = const.tile([P, D], bf16, tag=f"wo{i}")
            nc.sync.dma_start(out=t, in_=wo_d[i * P:(i + 1) * P, :])
            wo.append(t)
        tri3 = tri_sb.rearrange("p (h q) -> p h q", h=2)
        ones_sb = const.tile([P, DK], bf16, tag="ones")
        nc.vector.memset(ones_sb, 1.0)
        # HAM warm-up: bridge PE activity until the first input DMAs land so
        # the real matmuls start at the full 2.4 GHz clock.
        warm = const.tile([P, SBLK], bf16, tag="warm")
        nc.vector.memset(warm, 1.0)
        wps = psB.tile([P, SBLK], f32, tag="acc")
        for _ in range(14):
            nc.tensor.matmul(wps, warm[:, 0:P], warm,
                             start=True, stop=True)

        # ---- projection / outproj units, split into ~2-matmul pieces --------
        qt = [[None] * (nsb // 2) for _ in range(NPAIR)]   # [128, 1024]
        kt = [[None] * (nsb // 2) for _ in range(NPAIR)]
        vA = [[None] * NPAIR for _ in range(s // P)]
        vB = [[None] * NPAIR for _ in range(s // P)]

        unit_pieces = {}   # key -> deque of closures
        order = deque()    # unit keys in demand order

        def reg_unit(key, plist):
            if key in unit_pieces:
                return
            unit_pieces[key] = deque(plist)
            order.append(key)

        def qk_unit(is_q, pr, sbp, half):
            st = {}
            dests = qt if is_q else kt
            nm = ("qt" if is_q else "kt") + f"{pr}_{sbp}"
            wcol = (pr if is_q else NPAIR + pr) * P
            ct = tabs["cosq"] if is_q else tabs["cosk"]
            stt = tabs["sinq"] if is_q else tabs["sink"]
            cb = 2 * sbp + half
            c0 = cb * SBLK

            def mm(d0):
                def go():
                    if d0 == 0:
                        st['ps'] = psB.tile([P, SBLK], f32, tag="acc", name=f"ps{nm}")
                    for d in (d0, d0 + 1):
                        nc.tensor.matmul(
                            st['ps'], wq[d][:, wcol:wcol + P],
                            xT[d][:, c0:c0 + SBLK],
                            start=(d == 0), stop=(d == dch - 1))
                return go

            def fin():
                if dests[pr][sbp] is None:
                    dests[pr][sbp] = const.tile(
                        [P, W2], bf16, tag=nm, name=nm)
                dest = dests[pr][sbp]
                ps = st['ps']
                y = rpool.tile([P, SBLK], bf16, tag="y")
                nc.vector.tensor_copy(y, ps)
                sw = psB.tile([P, SBLK], f32, tag="acc")
                nc.tensor.matmul(sw, swap_sb, y, start=True, stop=True)
                t1 = rpool.tile([P, SBLK], bf16, tag="t1")
                nc.vector.tensor_mul(t1, y, ct[:, c0:c0 + SBLK])
                t2 = rpool.tile([P, SBLK], bf16, tag="t2")
                nc.vector.tensor_mul(t2, sw, stt[:, c0:c0 + SBLK])
                sl = slice(half * SBLK, (half + 1) * SBLK)
                nc.vector.tensor_add(dest[:, sl], t1, t2)

            return [mm(0), mm(2), mm(4), mm(6), fin]

        def v_unit(sc):
            st = {}

            def mm(d0):
                def go():
                    if d0 == 0:
                        st['ps'] = psB.tile([P, OV], f32, tag="acc", name=f"psv{sc}")
                    for d in (d0, d0 + 1):
                        nc.tensor.matmul(
                            st['ps'],
                            xT[d][:, sc * P:(sc + 1) * P],
                            wq[d][:, 2 * OV:],
                            start=(d == 0), stop=(d == dch - 1))
                return go

            def fin():
                psv = st['ps'].rearrange(
                    "p (a two d) -> p a two d", two=2, d=DK)
                va = const.tile([P, NPAIR, 65], bf16, tag=f"vA{sc}")
                vb = const.tile([P, NPAIR, P], bf16, tag=f"vB{sc}")
                nc.vector.tensor_copy(va[:, :, 0:DK], psv[:, :, 0, :])
                nc.gpsimd.memset(va[:, :, DK:DK + 1], 1.0)
                nc.gpsimd.memset(vb[:, :, 0:32], 0.0)
                nc.gpsimd.memset(vb[:, :, 32:33], 1.0)
                nc.gpsimd.memset(vb[:, :, 33:DK], 0.0)
                nc.vector.tensor_copy(vb[:, :, DK:2 * DK], psv[:, :, 1, :])
                for pr in range(NPAIR):
                    vA[sc][pr] = va[:, pr, :]
                    vB[sc][pr] = vb[:, pr, :]

            return [mm(0), mm(2), mm(4), mm(6), fin]

        def op_unit(qb, qc, nb):
            st = {}
            q_glob = qb * SBLK + qc * P
            nsl = slice(nb * SBLK, (nb + 1) * SBLK)

            def mm(pr0):
                def go():
                    if pr0 == 0:
                        st['po'] = psB.tile([P, SBLK], f32, tag="acc", name=f"po{qb}_{qc}_{nb}")
                    for pr2 in (pr0, pr0 + 1):
                        nc.tensor.matmul(
                            st['po'],
                            at[qb][pr2][:, qc * P:(qc + 1) * P],
                            wo[pr2][:, nsl],
                            start=(pr2 == 0), stop=(pr2 == NPAIR - 1))
                return go

            def fin():
                osb = opool.tile([P, SBLK], f32, tag="osb")
                nc.vector.tensor_copy(osb, st['po'])
                nc.sync.dma_start(out=out_d[q_glob:q_glob + P, nsl], in_=osb)

            return [mm(0), mm(2), fin]

        # demand-ordered registration of all projection units; v units are
        # registered one qb ahead so their DVE fins land well before the
        # first attnV matmul that streams them
        for qb in range(nsb):
            for sc in range(4 * qb, min(4 * qb + 8, s // P)):
                reg_unit(("v", sc), v_unit(sc))
            for pr in range(NPAIR):
                reg_unit(("q", pr, qb // 2, qb % 2),
                         qk_unit(True, pr, qb // 2, qb % 2))
                for kcb in range(qb + 1):
                    reg_unit(("k", pr, kcb // 2, kcb % 2),
                             qk_unit(False, pr, kcb // 2, kcb % 2))

        def pump_one():
            while order:
                key = order[0]
                dq = unit_pieces.get(key)
                if not dq:
                    order.popleft()
                    unit_pieces.pop(key, None)
                    continue
                dq.popleft()()
                if not dq:
                    order.popleft()
                    del unit_pieces[key]
                return True
            return False

        def run_unit(key):
            dq = unit_pieces.get(key)
            if dq is None:
                return
            # finish any half-open unit first so at most one proj psum
            # accumulator is live at a time
            while order and order[0] != key:
                fk = order[0]
                fdq = unit_pieces.get(fk)
                if fdq and len(fdq) < len_total.get(fk, 99):
                    while fdq:
                        fdq.popleft()()
                    order.popleft()
                    del unit_pieces[fk]
                else:
                    break
            while dq:
                dq.popleft()()
            unit_pieces.pop(key, None)
            if order and order[0] == key:
                order.popleft()

        len_total = {k: len(v) for k, v in unit_pieces.items()}

        def qt_sl(pr, qb, lo, hi, rows):
            t = qt[pr][qb // 2]
            off = (qb % 2) * SBLK
            return t[rows[0]:rows[1]][:, off + lo:off + hi]

        def kt_sl(pr, kc, r0, r1):
            t = kt[pr][kc // 8]
            off = (kc % 8) * P
            return t[r0:r1, off:off + P]

        # ---- attention ------------------------------------------------------
        at = [[None] * NPAIR for _ in range(nsb)]

        def emit_norm(qb, pr, accA, accB):
            rtb = npool.tile([P, SBLK], bf16, tag="recipb")
            with nc.allow_low_precision(reason="bf16 softmax denom"):
                nc.vector.tensor_copy(
                    rtb[DK:DK + 1, :], accA[DK:DK + 1, :])
                nc.vector.tensor_copy(rtb[32:33, :], accB[32:33, :])
            # plain-MM filler so the broadcast matmuls below never wait on
            # the DVE copies at the PE queue head; fall back to warm-keeper
            # matmuls when no real work is left (keeps HAM at full clock
            # through the final norm chains)
            filled = pump_one() + pump_one()
            if filled < 2:
                wpsn = psB.tile([P, SBLK], f32, tag="acc",
                                name=f"wnorm{qb}_{pr}")
                for _ in range(6):
                    nc.tensor.matmul(wpsn, warm[:, 0:P], warm,
                                     start=True, stop=True)
            rbp = psS.tile([P, W2], f32, tag="mm")
            nc.tensor.matmul(
                rbp[0:DK, 0:SBLK], ones_sb[DK:DK + 1, :],
                rtb[DK:DK + 1, :],
                start=True, stop=True, tile_position=(64, 0))
            nc.tensor.matmul(
                rbp[DK:P, 0:SBLK], ones_sb[32:33, :], rtb[32:33, :],
                start=True, stop=True, tile_position=(32, 64))
            rbs = npool.tile([P, SBLK], f32, tag="rbcast")
            nc.vector.reciprocal_approx_fast(rbs, rbp[:, 0:SBLK])
            atile = atpool.tile([P, SBLK], bf16, tag=f"at{pr}")
            nc.vector.tensor_tensor(
                atile[0:DK, :], accA[0:DK, :], rbs[0:DK, :], op=MUL)
            nc.vector.tensor_tensor(
                atile[DK:P, :], accB[DK:P, :], rbs[DK:P, :], op=MUL)
            at[qb][pr] = atile

        def drain2(pending, accA, accB, pr, nkc):
            # drain 2 pending chunks' attnV (4 plain MMs, adjacent)
            for _ in range(2):
                pkc, ppp, pq0 = pending.pop(0)
                nc.tensor.matmul(
                    accA[0:65, pq0:SBLK], vA[pkc][pr],
                    ppp[:, pq0:SBLK],
                    start=(pkc == 0), stop=(pkc == nkc - 1))
                nc.tensor.matmul(
                    accB[0:P, pq0:SBLK], vB[pkc][pr],
                    ppp[:, SBLK + pq0:W2],
                    start=(pkc == 0), stop=(pkc == nkc - 1))

        def finish_pair(pend):
            # drain the deferred attnV tail (exp caught up during the burst
            # above), then normalize; register outproj once a qb completes
            pqb, ppr, paccA, paccB, ppending = pend
            pnkc = 4 * pqb + 4
            while ppending:
                drain2(ppending, paccA, paccB, ppr, pnkc)
            emit_norm(pqb, ppr, paccA, paccB)
            if ppr == NPAIR - 1:
                for qc in range(SBLK // P):
                    for nb in range(D // SBLK):
                        key = ("o", pqb, qc, nb)
                        unit_pieces[key] = deque(op_unit(pqb, qc, nb))
                        len_total[key] = len(unit_pieces[key])
                        order.appendleft(key)

        pend = None
        for qb in range(nsb):
            for pr in range(NPAIR):
                # ensure this pair's projection inputs are complete
                for sc in range(4 * qb, 4 * qb + 4):
                    run_unit(("v", sc))
                run_unit(("q", pr, qb // 2, qb % 2))
                for kcb in range(qb + 1):
                    run_unit(("k", pr, kcb // 2, kcb % 2))
                if pend is not None:
                    if qb < 2:
                        # short pairs: bridge the exp-catchup latency with
                        # filler so the drain below never stalls the PE
                        for _ in range(3):
                            pump_one()
                    finish_pair(pend)
                    pend = None
                accA = psB.tile([P, SBLK], f32, tag="acc")
                accB = psB.tile([P, SBLK], f32, tag="acc")
                nkc = 4 * qb + 4
                pending = []
                # 2-chunk groups: scores pairs adjacent (one tiled burst),
                # then exps, then plain pump + lagged attnV (plain burst) --
                # halves the PE tiled<->plain reconfig bubbles
                for g in range(nkc // 2):
                    descs = []
                    for c in (0, 1):
                        kc = 2 * g + c
                        diag_o = kc - 4 * qb
                        q0 = max(diag_o, 0) * P
                        sp = psS.tile([P, W2], f32, tag="mm")
                        nc.tensor.matmul(
                            sp[:, q0:SBLK],
                            kt_sl(pr, kc, 0, DK),
                            qt_sl(pr, qb, q0, SBLK, (0, DK)),
                            start=True, stop=True, tile_position=(0, 0))
                        nc.tensor.matmul(
                            sp[:, SBLK + q0:W2],
                            kt_sl(pr, kc, DK, P),
                            qt_sl(pr, qb, q0, SBLK, (DK, P)),
                            start=True, stop=True, tile_position=(64, 0))
                        descs.append((kc, sp, q0, diag_o))
                    for kc, sp, q0, diag_o in descs:
                        pp = ppool.tile([P, W2], bf16, tag="p")
                        if diag_o < 0:
                            nc.scalar.activation(pp, sp, EXP)
                        else:
                            nc.scalar.activation(
                                pp.rearrange(
                                    "p (h q) -> p h q", h=2)[:, :, q0:SBLK],
                                sp.rearrange(
                                    "p (h q) -> p h q", h=2)[:, :, q0:SBLK],
                                EXP)
                        if diag_o >= 0:
                            sl = pp.rearrange(
                                "p (h q) -> p h q", h=2)[:, :, q0:q0 + P]
                            nc.vector.tensor_tensor(sl, sl, tri3, op=MUL)
                        pending.append((kc, pp, q0))
                    # plain filler between the tiled scores burst and the
                    # attnV burst; keeps LDWEIGHTS hidden + exp time covered
                    pump_one()
                    pump_one()
                    if len(pending) >= 6:
                        drain2(pending, accA, accB, pr, nkc)
                pend = (qb, pr, accA, accB, pending)
        # keep the PE clock warm across the final norm chain latency
        wps2 = psB.tile([P, SBLK], f32, tag="acc", name="warmtail")
        for _ in range(6):
            nc.tensor.matmul(wps2, warm[:, 0:P], warm,
                             start=True, stop=True)
        finish_pair(pend)
        for _ in range(8):
            nc.tensor.matmul(wps2, warm[:, 0:P], warm,
                             start=True, stop=True)
        while pump_one():
            pass

    nc.finalize()
    return nc


def _host_prep(x, Wqkv, Wout, s=S):
    perm = _rope_perm()
    cosq, sinq, cosk, sink, swapm, tri2 = _host_tables(s)
    in_maps = []
    for c in range(NCORES):
        b, hh = c // 2, c % 2
        rows = []
        for sect in range(3):
            base = sect * D + hh * OV
            for h in range(HLOC):
                r = base + h * DK + (perm if sect < 2 else np.arange(DK))
                rows.append(r)
        idx = np.concatenate(rows)
        wslice = Wqkv[idx, :]
        in_maps.append({
            "xT": np.ascontiguousarray(x[b].T).astype(BF16),
            "wqkvT": np.ascontiguousarray(wslice.T).astype(BF16),
            "woutT": np.ascontiguousarray(
                Wout[:, hh * OV:(hh + 1) * OV].T).astype(BF16),
            "cosk": cosk, "sink": sink,
            "swapm": swapm, "tri2": tri2,
        })
    return in_maps


def kernel(x, Wqkv, Wout):
    from concourse.bass_utils import run_bass_kernel_spmd

    x = np.asarray(x, dtype=np.float32)
    Wqkv = np.asarray(Wqkv, dtype=np.float32)
    Wout = np.asarray(Wout, dtype=np.float32)

    nc = _build_nc(S)
    in_maps = _host_prep(x, Wqkv, Wout, S)
    res = run_bass_kernel_spmd(nc, in_maps, core_ids=list(range(NCORES)))
    outs = res.results
    out = np.empty((B, S, D), np.float32)
    for b in range(B):
        out[b] = outs[2 * b]["out"] + outs[2 * b + 1]["out"]
    return out

